# revision 31
# baseline (speedup 1.0000x reference)
"""Bahdanau additive attention on 8 Trainium2 NeuronCores.

Data-parallel over batch: core c handles batches [4c, 4c+4).
Per batch b (tcc-outer over two 512-col t-chunks):
  ep[k,t]   = sum_h Ua[k,h] * enc[b,t,h]        (bf16 PE matmuls, k on PSUM partitions)
  z[k,t]    = tanh(ep[k,t] + hp[b,k])           (ScalarE, hp as per-partition bias)
  e[t]      = sum_k va[k] * z[k,t]              (M=1 bf16 PE matmuls into 4 col-packed
                                                 e_tile rows; per-chunk accumulation)
  exp rows  = exp(e) out of PSUM (ScalarE) -> mask mult (DVE) -> PE broadcast matmul
              ones128^T x exm -> bc PSUM [128, 512] -> cast to bf16 (ScalarE)
  ctx       = DVE tensor_tensor_reduce: (encT[:,ht,:] * bc_b) accumulated per ht into
              ctxT [128, 8] across both chunks; softmax 1/sum folded at the end
              (per-partition rinv via free-axis reduce of bc_b).
No second enc copy: ctx reuses the encT tiles already resident for ep.
hp[b,k] = sum_h Wa[k,h] * h_t[b,h] runs as per-kt N=4 matmul chains in the prologue.
Weights ship in one packed [128, WX] DRAM tensor; prologue DMA issues are spread
across the sync/vector/scalar/gpsimd queues (descriptor issue costs ~0.65us each).
"""

import numpy as np

import concourse.bass as bass
import concourse.tile as tile
from concourse import bacc, mybir

dt = mybir.dt
AF = mybir.ActivationFunctionType
ALU = mybir.AluOpType
AXX = mybir.AxisListType.X

B, T, H = 32, 1024, 1024
NCORES = 8
BL = B // NCORES          # batches per core
P = 128                   # partitions
NT = 512                  # matmul free-dim chunk (one PSUM bank of fp32)
KT = H // P               # k-tiles (output rows of ep)
HT = H // P               # h-tiles (contraction)
TC = T // NT              # t chunks per batch
EC = 4                    # e col-packed rows (one per 256-col strip)
EW = T // EC              # 256

# packed-weight column offsets (bf16 elements per partition)
OFF_UA0 = 0
OFF_VA = 1024
OFF_HTT = 1032
OFF_WA0 = 1064
OFF_REST = 2088           # kt>=1: [uaT_kt (1024) | waT_kt (1024)] blocks
WX = OFF_REST + (KT - 1) * 2048


def _off_ua(kt):
    return OFF_UA0 if kt == 0 else OFF_REST + (kt - 1) * 2048


def _off_wa(kt):
    return OFF_WA0 if kt == 0 else OFF_REST + (kt - 1) * 2048 + 1024


_CACHE = {}


def _build_nc():
    nc = bacc.Bacc("TRN2", target_bir_lowering=False, debug=False)

    encT_d = nc.dram_tensor("encT", [BL, TC, P, HT, NT], dt.bfloat16,
                            kind="ExternalInput").ap()
    wall_d = nc.dram_tensor("wall", [P, WX], dt.bfloat16,
                            kind="ExternalInput").ap()
    mask_d = nc.dram_tensor("mask_m1", [BL, T], dt.bfloat16,
                            kind="ExternalInput").ap()

    # ctx ships transposed [P, KT] per batch (h = ht*128 + p); host reassembles
    ctx_d = nc.dram_tensor("ctx", [BL, P, KT], dt.float32,
                           kind="ExternalOutput").ap()
    attn_d = nc.dram_tensor("attn", [BL, T], dt.float32, kind="ExternalOutput").ap()
    ctx_v = ctx_d

    with tile.TileContext(nc) as tc:
        from contextlib import ExitStack

        with ExitStack() as st:
            wpool = st.enter_context(tc.tile_pool(name="weights", bufs=1))
            etpool = st.enter_context(tc.tile_pool(name="encT", bufs=4))
            thpool = st.enter_context(tc.tile_pool(name="tanh", bufs=4))
            smpool = st.enter_context(tc.tile_pool(name="small", bufs=1))
            pmain = st.enter_context(tc.tile_pool(name="pmain", bufs=4, space="PSUM"))
            pe_ps = st.enter_context(tc.tile_pool(name="pe", bufs=2, space="PSUM"))
            pbc = st.enter_context(tc.tile_pool(name="pbc", bufs=2, space="PSUM"))

            wall_sb = wpool.tile([P, WX], dt.bfloat16, tag="wall")
            mask_sb = wpool.tile([1, BL * T], dt.bfloat16, tag="mask")

            def uaT_ap(kt, ht):
                o = _off_ua(kt) + ht * P
                return wall_sb[:, o:o + P]

            def waT_ap(kt, ht):
                o = _off_wa(kt) + ht * P
                return wall_sb[:, o:o + P]

            def htT_ap(ht):
                o = OFF_HTT + ht * BL
                return wall_sb[:, o:o + BL]

            def va_ap(kt):
                return wall_sb[:, OFF_VA + kt:OFF_VA + kt + 1]

            def load_encT(bi, tcc, eng):
                t_ = etpool.tile([P, HT, NT], dt.bfloat16, tag="encT",
                                 name=f"encT{bi}_{tcc}")
                eng.dma_start(t_[:], encT_d[bi, tcc])
                return t_

            # prologue DMAs: need-ordered, spread over the 3 issue queues
            nc.sync.dma_start(wall_sb[:, OFF_UA0:OFF_WA0],
                              wall_d[:, OFF_UA0:OFF_WA0])     # uaT0+va+htT
            nc.gpsimd.dma_start(wall_sb[:, OFF_WA0:OFF_REST],
                                wall_d[:, OFF_WA0:OFF_REST])  # waT0
            encT_cur = [load_encT(0, 0, nc.scalar)]
            encT_cur.append(load_encT(0, 1, nc.scalar))
            for _kt in range(1, KT):
                eng = nc.sync if _kt % 2 else nc.gpsimd
                c0 = _off_ua(_kt)
                c1 = WX if _kt == KT - 1 else _off_ua(_kt + 1)
                eng.dma_start(wall_sb[:, c0:c1], wall_d[:, c0:c1])
            nc.gpsimd.dma_start(mask_sb[:], mask_d.rearrange("b t -> (b t)"))

            # PE warm-up: back-to-back dummy matmuls while the prologue DMAs
            # stream in, so HAM ramps before the first real group.
            ones_b = wpool.tile([1, P], dt.bfloat16, tag="ones_b")
            nc.vector.memset(ones_b[:], 1.0)
            hp_sb = wpool.tile([P, KT, BL], dt.float32, tag="hp")
            g_rhs = wpool.tile([P, NT], dt.bfloat16, tag="g_rhs")
            nc.vector.memset(g_rhs[:], 0.0)
            warm_ps = pbc.tile([P, NT], dt.float32, tag="bc", name="warm_ps")
            for _ in range(16):
                nc.tensor.matmul(warm_ps[:], g_rhs[:, :P], g_rhs[:],
                                 start=True, stop=True)

            # ---- deferred post-op FIFO: two items popped per main group ----
            post_q = []

            def pop_post(n=2):
                for _ in range(n):
                    if post_q:
                        post_q.pop(0)()

            def make_echain(e_tile, bi, tcc, ths, half):
                # contiguous accumulation chains per chunk: stream at full PE
                # rate instead of paying per-group fill/drain. The first half
                # opens with the additive mask row ((m-1)*1e30) so exp masks
                # for free; the second half closes the accumulation.
                def emit():
                    row = e_tile[64 * tcc:64 * tcc + 1, :]
                    tp = (0, 64 * tcc)
                    if half == 0:
                        nc.tensor.matmul(
                            row, ones_b[0:1, 0:1],
                            mask_sb[:, bi * T + tcc * NT:
                                    bi * T + (tcc + 1) * NT],
                            start=True, stop=False, tile_position=tp)
                    for kt in range(half * 4, half * 4 + 4):
                        nc.tensor.matmul(
                            row, va_ap(kt), ths[kt][:],
                            start=False, stop=(kt == KT - 1),
                            tile_position=tp)
                return emit

            def make_exp(tcc, e_tile, ex):
                def emit():
                    sl = slice(tcc * NT, (tcc + 1) * NT)
                    nc.scalar.activation(ex[:, sl],
                                         e_tile[64 * tcc:64 * tcc + 1, :],
                                         AF.Exp)
                return emit

            def make_bc(tcc, ex, boxes):
                def emit():
                    bc = pbc.tile([P, NT], dt.float32, tag="bc",
                                  name=f"bc{tcc}")
                    boxes[f"bc{tcc}"] = bc
                    nc.tensor.matmul(bc[:], ones_b[:],
                                     ex[:, tcc * NT:(tcc + 1) * NT],
                                     start=True, stop=True)
                return emit

            def make_cast(tcc, boxes, ssc):
                def emit():
                    bcb = smpool.tile([P, NT], dt.bfloat16, tag="bcb", bufs=2,
                                      name=f"bcb{tcc}")
                    boxes[f"bcb{tcc}"] = bcb
                    # cast PSUM->SBUF bf16 and accumulate the softmax
                    # denominator in the same ScalarE pass
                    nc.scalar.activation(bcb[:], boxes[f"bc{tcc}"][:], AF.Copy,
                                         accum_out=ssc[:, tcc:tcc + 1])
                return emit

            # ctx reduction: (encT * bcb) summed along free axis per ht.
            # 4 tiles on DVE via fused scalar_tensor_tensor(accum_out); 4 tiles
            # via GpSimd tensor_mul feeding ScalarE Copy-with-accum_out.
            # (tensor_tensor_reduce wedges the device — HW-verified; gpsimd
            # cannot reduce along the free axis.)
            def make_ctx(tcc, boxes, encT_t, ctxT0, ctxT1, scr_v):
                def emit():
                    bcb = boxes[f"bcb{tcc}"]
                    ctxc = ctxT0 if tcc == 0 else ctxT1
                    for ht in range(HT):
                        nc.vector.scalar_tensor_tensor(
                            scr_v[:], encT_t[tcc][:, ht, :], 1.0, bcb[:],
                            ALU.mult, ALU.mult,
                            accum_out=ctxc[:, ht:ht + 1])
                return emit

            def make_finish1(bi, ex, ssc, boxes):
                def emit():
                    sst = smpool.tile([P, 1], dt.float32, tag="sst", bufs=2)
                    nc.vector.tensor_add(sst[:], ssc[:, 0:1], ssc[:, 1:2])
                    rinv = smpool.tile([P, 1], dt.float32, tag="rinv", bufs=2)
                    boxes["rinv"] = rinv
                    nc.vector.reciprocal(rinv[:], sst[:])
                    attn_sb = smpool.tile([1, T], dt.float32, tag="attn", bufs=2)
                    for tcc in range(TC):
                        nc.scalar.mul(attn_sb[:, tcc * NT:(tcc + 1) * NT],
                                      ex[:, tcc * NT:(tcc + 1) * NT],
                                      rinv[0:1, 0:1])
                    nc.gpsimd.dma_start(attn_d[bi:bi + 1, :], attn_sb[:])
                return emit

            def make_finish2(bi, ctxT0, ctxT1, boxes):
                def emit():
                    rinv = boxes["rinv"]
                    ctxf = smpool.tile([P, KT], dt.float32, tag="ctxf", bufs=2)
                    nc.vector.tensor_add(ctxT1[:], ctxT0[:], ctxT1[:])
                    nc.vector.tensor_scalar_mul(ctxf[:], ctxT1[:], rinv[:])
                    nc.gpsimd.dma_start(ctx_v[bi], ctxf[:])
                return emit

            # seed the queue with warm filler matmuls: popped between the
            # DMA-gated prologue groups they plug PE idle holes so the HAM
            # activity window stays busy and the clock un-throttles early
            def make_filler():
                def emit():
                    wps = pmain.tile([P, NT], dt.float32, tag="big",
                                     name="warmf")
                    for _ in range(2):
                        nc.tensor.matmul(wps[:], g_rhs[:, :P], g_rhs[:],
                                         start=True, stop=True)
                return emit

            for _ in range(8):
                post_q.append(make_filler())

            # ---- main loop: tcc-outer everywhere ----
            for bi in range(BL):
                e_tile = pe_ps.tile([P, NT], dt.float32, tag="e",
                                    name=f"e_ps{bi}")
                ex = smpool.tile([1, T], dt.bfloat16, tag="ex", bufs=2,
                                 name=f"ex{bi}")
                ssc = smpool.tile([P, TC], dt.float32, tag="ssc", bufs=2,
                                  name=f"ssc{bi}")
                ctxT0 = smpool.tile([P, KT], dt.float32, tag="ctxT0", bufs=2,
                                    name=f"ctxT0_{bi}")
                ctxT1 = smpool.tile([P, KT], dt.float32, tag="ctxT1", bufs=2,
                                    name=f"ctxT1_{bi}")
                scr_v = smpool.tile([P, NT], dt.bfloat16, tag="scr_v", bufs=1,
                                    name="scr_v")
                boxes = {}
                ths = {}
                for gi in range(TC * KT):
                    tcc, kt = gi // KT, gi % KT
                    # prefetch next batch's encT tiles
                    if bi < BL - 1:
                        if gi == 5:
                            encT_next = [load_encT(bi + 1, 0, nc.sync)]
                        elif gi == 10:
                            encT_next.append(load_encT(bi + 1, 1, nc.sync))
                    ps = pmain.tile([P, NT], dt.float32, tag="big")
                    for ht in range(HT):
                        nc.tensor.matmul(
                            ps[:], uaT_ap(kt, ht), encT_cur[tcc][:, ht, :],
                            start=(ht == 0), stop=(ht == HT - 1))
                    if bi == 0 and tcc == 0:
                        hp_ps = pbc.tile([P, NT], dt.float32, tag="bc",
                                         name=f"hp_ps{kt}")
                        for ht in range(HT):
                            nc.tensor.matmul(
                                hp_ps[:, 0:BL], waT_ap(kt, ht), htT_ap(ht),
                                start=(ht == 0), stop=(ht == HT - 1))
                        nc.vector.tensor_copy(hp_sb[:, kt, :], hp_ps[:, 0:BL])
                    th = thpool.tile([P, NT], dt.bfloat16, tag="th",
                                     bufs=11, name="th")
                    nc.scalar.activation(th[:], ps[:], AF.Tanh,
                                         bias=hp_sb[:, kt, bi:bi + 1])
                    ths[kt] = th
                    pop_post()
                    if kt == 3:
                        post_q.append(make_echain(e_tile, bi, tcc,
                                                  dict(ths), 0))
                    if kt == KT - 1:
                        post_q.append(make_echain(e_tile, bi, tcc,
                                                  dict(ths), 1))
                        post_q.append(make_exp(tcc, e_tile, ex))
                        post_q.append(make_bc(tcc, ex, boxes))
                        post_q.append(make_cast(tcc, boxes, ssc))
                        if tcc == TC - 1:
                            post_q.append(make_finish1(bi, ex, ssc, boxes))
                        post_q.append(make_ctx(tcc, boxes, encT_cur,
                                               ctxT0, ctxT1, scr_v))
                        if tcc == TC - 1:
                            post_q.append(make_finish2(bi, ctxT0, ctxT1,
                                                       boxes))
                if bi < BL - 1:
                    encT_cur = encT_next
            while post_q:
                post_q.pop(0)()

    nc.compile()
    return nc


def _get_runner():
    if "runner" in _CACHE:
        return _CACHE["runner"]

    import jax
    from jax.sharding import Mesh, PartitionSpec
    from jax.experimental.shard_map import shard_map
    from concourse import bass2jax
    from concourse import mybir as _mb

    nc = _build_nc()
    bass2jax.install_neuronx_cc_hook()

    partition_name = (nc.partition_id_tensor.name
                      if nc.partition_id_tensor else None)
    in_names, out_names, out_avals, zero_outs = [], [], [], []
    for alloc in nc.m.functions[0].allocations:
        if not isinstance(alloc, _mb.MemoryLocationSet):
            continue
        name = alloc.memorylocations[0].name
        if alloc.kind == "ExternalInput":
            if name != partition_name:
                in_names.append(name)
        elif alloc.kind == "ExternalOutput":
            out_names.append(name)
            shape = tuple(alloc.tensor_shape)
            npdt = _mb.dt.np(alloc.dtype)
            out_avals.append(jax.core.ShapedArray(shape, npdt))
            zero_outs.append(np.zeros(shape, npdt))
    n_params = len(in_names)
    n_outs = len(out_names)
    all_in_names = in_names + out_names
    if partition_name is not None:
        all_in_names = all_in_names + [partition_name]
    import os as _os
    if _os.environ.get("KERNEL_SIM"):
        donate = ()
    else:
        donate = tuple(range(n_params, n_params + n_outs))

    def _body(*args):
        operands = list(args)
        if partition_name is not None:
            operands.append(bass2jax.partition_id_tensor())
        outs = bass2jax._bass_exec_p.bind(
            *operands,
            out_avals=tuple(out_avals),
            in_names=tuple(all_in_names),
            out_names=tuple(out_names),
            lowering_input_output_aliases=(),
            sim_require_finite=True,
            sim_require_nnan=True,
            nc=nc,
        )
        return tuple(outs)

    devices = jax.devices()[:NCORES]
    mesh = Mesh(np.asarray(devices), ("core",))
    in_specs = (PartitionSpec("core"),) * (n_params + n_outs)
    out_specs = (PartitionSpec("core"),) * n_outs
    sharded = jax.jit(
        shard_map(_body, mesh=mesh, in_specs=in_specs, out_specs=out_specs,
                  check_rep=False),
        donate_argnums=donate, keep_unused=True)

    def run(in_maps):
        concat_in = [
            np.concatenate([np.asarray(m[name]) for m in in_maps], axis=0)
            for name in in_names
        ]
        concat_zeros = [
            np.zeros((NCORES * z.shape[0], *z.shape[1:]), z.dtype)
            for z in zero_outs
        ]
        out_arrs = sharded(*concat_in, *concat_zeros)
        return [
            {name: np.asarray(out_arrs[i]).reshape(NCORES, *out_avals[i].shape)[c]
             for i, name in enumerate(out_names)}
            for c in range(NCORES)
        ]

    _CACHE["runner"] = run
    return run


def _make_in_maps(inputs):
    import ml_dtypes
    bf16 = ml_dtypes.bfloat16

    h_t = np.asarray(inputs["h_t"], dtype=np.float32)
    enc_out = np.asarray(inputs["enc_out"], dtype=np.float32)
    src_mask = np.asarray(inputs["src_mask"])
    Wa = np.asarray(inputs["Wa"], dtype=np.float32)
    Ua = np.asarray(inputs["Ua"], dtype=np.float32)
    va = np.asarray(inputs["va"], dtype=np.float32)

    # [KT, P, HT, P] column blocks of Ua.T / Wa.T (lhsT layouts)
    uaT = np.ascontiguousarray(
        Ua.T.reshape(HT, P, KT, P).transpose(2, 1, 0, 3)).astype(bf16)
    waT = np.ascontiguousarray(
        Wa.T.reshape(HT, P, KT, P).transpose(2, 1, 0, 3)).astype(bf16)
    va_pk = np.ascontiguousarray(va.reshape(KT, P).T).astype(bf16)   # [P, KT]
    encT = np.ascontiguousarray(
        enc_out.transpose(0, 2, 1).reshape(B, HT, P, TC, NT)
        .transpose(0, 3, 2, 1, 4)).astype(bf16)                 # [B, TC, P, HT, NT]
    mask_m1 = np.ascontiguousarray(
        (src_mask.astype(np.float32) - 1.0) * 1e30).astype(bf16)

    in_maps = []
    for c in range(NCORES):
        sl = slice(c * BL, (c + 1) * BL)
        htT = np.ascontiguousarray(
            h_t[sl].T.reshape(HT, P, BL).transpose(1, 0, 2)
            .reshape(P, HT * BL)).astype(bf16)                  # [P, HT*BL]
        wall = np.empty((P, WX), dtype=bf16)
        wall[:, OFF_UA0:OFF_VA] = uaT[0].reshape(P, HT * P)
        wall[:, OFF_VA:OFF_HTT] = va_pk
        wall[:, OFF_HTT:OFF_WA0] = htT
        wall[:, OFF_WA0:OFF_REST] = waT[0].reshape(P, HT * P)
        for kt in range(1, KT):
            o = OFF_REST + (kt - 1) * 2048
            wall[:, o:o + 1024] = uaT[kt].reshape(P, HT * P)
            wall[:, o + 1024:o + 2048] = waT[kt].reshape(P, HT * P)
        in_maps.append({
            "encT": encT[sl],
            "wall": wall,
            "mask_m1": mask_m1[sl],
        })
    return in_maps


def kernel(h_t, enc_out, src_mask, Wa, Ua, va):
    in_maps = _make_in_maps({
        "h_t": h_t, "enc_out": enc_out, "src_mask": src_mask,
        "Wa": Wa, "Ua": Ua, "va": va,
    })
    run = _get_runner()
    results = run(in_maps)
    context = np.concatenate(
        [r["ctx"].transpose(0, 2, 1).reshape(BL, H) for r in results], axis=0)
    attn = np.concatenate([r["attn"] for r in results], axis=0)
    return context, attn
